# revision 2
# baseline (speedup 1.0000x reference)
"""Trainium2 Bass kernel for MultiHeadSelfAttention with ALiBi + adjacency bias.

Sharding: 8 cores = 2 batches x 4 pair-groups. Core c (b=c//4, a=c%4) owns
heads [2a, 2a+1, 8+2a, 9+2a]: pair0 = ALiBi heads (slopes 2^-(h+1)),
pair1 = flat heads (slope 0).

Design (all matmuls bf16):
  A) qkvT[c, l] = (W_qk^T @ X^T) (transposed, head-major cols, 1/8 folded
     into Q); V_sb[l, h, hs] = X @ W_v (+bias), masked by mask_k, plus a
     ones column per head -> V_aug lhsT [k, 65].
  B) Shared Ea = exp(gamma*adjT) bf16 [2048, 2048] SBUF-RESIDENT (8.4MB,
     loaded once) - replaces the per-head E DMA (was 33.5MB/core).
     ALiBi factor exp(-s|k-q|) decomposes per (qh, kb) tile:
       below-diag (k < q0):        exp(s(k-q0))     * exp(-s(q-q0))
       above-diag (k >= q0+512):   exp(-s(k-q0-511))* exp(s(q-q0-511))
     row part (per-partition k) -> folded into V via tensor_scalar [128,65];
     col part (per-q) -> applied on HOST: O accumulated in 3 PSUM phases
     (below/cross/above), each drained separately; host combines.
     Diagonal-crossing tiles use host-precomputed Ecross = Ea*exp(-s|k-q|)
     (bf16, streamed, 4.2MB/core).
  C) per head-pair, per (qh, kb): S^T[k,q] = K Q^T/8 in PSUM fp32
     (concurrent PE row tiles 0-63/64-127), pT = exp(S^T) on ACT
     (PSUM->SBUF bf16, one op for both heads), pb = pT * Ea (DVE bf16,
     broadcast AP reads the 512-wide Ea tile twice), O^T_aug[65,q] +=
     V_aug^T @ pb accumulated per phase; row 64 = denom.
  Host: combine phase partials with col factors, divide by denom, apply
  mask_q, transpose per-head, assemble, +out_bias.
"""

import math

import numpy as np

B, L, D = 2, 2048, 1024
NH, HS = 16, 64
HPC = 4          # heads per core
NKB = L // 128   # 16 k blocks
QW = 512         # q tile width (1 PSUM bank)
NQH = L // QW    # 4 q tiles

_cache = {}


def _alibi_slopes_full():
    ah = NH // 2
    start = 2.0 ** (-(2.0 ** -(math.log2(ah) - 3)))
    s = [start * (start ** i) for i in range(ah)]
    return np.array(s + [0.0] * (NH - ah), dtype=np.float32)


def _core_heads(c):
    a = c % HPC
    return [2 * a, 2 * a + 1, 8 + 2 * a, 9 + 2 * a]


def _build():
    import concourse.tile as tile
    import concourse.mybir as mybir
    from concourse import bacc
    from contextlib import ExitStack

    dt = mybir.dt
    F32, BF16 = dt.float32, dt.bfloat16
    Alu = mybir.AluOpType
    Act = mybir.ActivationFunctionType

    nc = bacc.Bacc("TRN2", target_bir_lowering=False, num_devices=8)

    # xT | wqk | wv concatenated: one DMA per 128-row chunk of D
    xw_d = nc.dram_tensor("xw", [D, L + 512 + 256], BF16, kind="ExternalInput")
    biasqk_d = nc.dram_tensor("biasqk", [128, 4], F32, kind="ExternalInput")
    biasv_d = nc.dram_tensor("biasv", [1, 256], BF16, kind="ExternalInput")
    mask16_d = nc.dram_tensor("mask16", [128, NKB], F32, kind="ExternalInput")
    ea_d = nc.dram_tensor("ea", [128, NKB * L], BF16, kind="ExternalInput")
    ecross_d = nc.dram_tensor(
        "ecross", [NQH, 4, 128, 2 * QW], BF16, kind="ExternalInput")
    rowfac_d = nc.dram_tensor(
        "rowfac", [128, NQH * NKB * 2], F32, kind="ExternalInput")
    oun_d = nc.dram_tensor("o_un", [HPC, 3, 65, L], F32, kind="ExternalOutput")

    with tile.TileContext(nc) as tc, ExitStack() as ctx:
        persist = ctx.enter_context(tc.tile_pool(name="persist", bufs=1))
        # Q^T,K^T bf16: mb 0-1 = Q pairs (h on part 0-63/64-127), 2-3 = K
        qkvT = persist.tile([128, 4, L], BF16)
        # V_aug: [k_part, kb, h, 66] - cols 0:64 = V*mask, col 64 = ones
        vsb = persist.tile([128, NKB, HPC, 66], BF16)
        # shared exp(gamma*adjT): [k_part, kb, q]
        ea = persist.tile([128, NKB, L], BF16)
        rowfac_sb = persist.tile([128, NQH * NKB * 2], F32)

        pa = ctx.enter_context(tc.tile_pool(name="phaseA", bufs=1))
        pe = ctx.enter_context(tc.tile_pool(name="pe", bufs=4))
        pp = ctx.enter_context(tc.tile_pool(name="pp", bufs=8))
        pq = ctx.enter_context(tc.tile_pool(name="pq", bufs=8))
        vp = ctx.enter_context(tc.tile_pool(name="vp", bufs=4))
        outp = ctx.enter_context(tc.tile_pool(name="outp", bufs=4))
        psA = ctx.enter_context(tc.tile_pool(name="psA", bufs=2, space="PSUM"))
        psS = ctx.enter_context(tc.tile_pool(name="psS", bufs=2, space="PSUM"))
        psO = ctx.enter_context(tc.tile_pool(name="psO", bufs=1, space="PSUM"))

        xw_r = pa.tile([128, D // 128, L + 512 + 256], BF16)
        xw_dv = xw_d.rearrange("(o p) c -> p o c", p=128)
        for kc in range(D // 128):
            nc.sync.dma_start(xw_r[:, kc, :], xw_dv[:, kc, :])
        biasqk_sb = pa.tile([128, 4], F32)
        nc.sync.dma_start(biasqk_sb[:], biasqk_d[:])
        biasv_sb = pa.tile([1, 256], BF16)
        nc.sync.dma_start(biasv_sb[:], biasv_d[:])
        mask_sb = pa.tile([128, NKB], F32)
        nc.sync.dma_start(mask_sb[:], mask16_d[:])
        nc.sync.dma_start(rowfac_sb[:], rowfac_d[:])
        # resident Ea: kb 4..15 first (first non-cross consumers), then 0..3
        for kb in list(range(4, NKB)) + list(range(4)):
            nc.sync.dma_start(ea[:, kb, :], ea_d[:, kb * L:(kb + 1) * L])
        ones1 = pa.tile([1, 128], BF16)
        nc.vector.memset(ones1[:], 1.0)
        nc.vector.memset(vsb[:, :, :, 64:65], 1.0)
        # tiny dummy exp: pulls the ~2.7us ACT_TABLE_LOAD into the DMA ramp
        wtmp = pa.tile([1, 16], F32)
        nc.vector.memset(wtmp[:], 0.0)
        wex = pa.tile([1, 16], BF16)
        nc.scalar.activation(wex[:], wtmp[:], Act.Exp)

        def t_group(mb, nqa, nqb):
            # qkvT[c, l] blocks (mb, nqa) and (mb, nqb) via two interleaved
            # accumulation chains on different PSUM banks sharing each
            # LDWEIGHTS -> fills/drains overlap instead of serializing.
            # Returns two emission chunks (for fine-grained interleaving).
            state = {}

            def half(lo, hi):
                if lo == 0:
                    state["psa"] = psA.tile([128, 512], F32, tag="psA", name="psa")
                    state["psb"] = psA.tile([128, 512], F32, tag="psA", name="psb")
                psa, psb = state["psa"], state["psb"]
                for kc in range(lo, hi):
                    w = xw_r[:, kc, L + mb * 128:L + (mb + 1) * 128]
                    nc.tensor.matmul(
                        psa[:], w, xw_r[:, kc, nqa * 512:(nqa + 1) * 512],
                        start=(kc == 0), stop=(kc == D // 128 - 1),
                    )
                    nc.tensor.matmul(
                        psb[:], w, xw_r[:, kc, nqb * 512:(nqb + 1) * 512],
                        start=(kc == 0), stop=(kc == D // 128 - 1),
                    )
                if hi == D // 128:
                    for nq, ps in ((nqa, psa), (nqb, psb)):
                        nc.vector.tensor_scalar(
                            qkvT[:, mb, nq * 512:(nq + 1) * 512], ps[:],
                            biasqk_sb[:, mb:mb + 1], None, Alu.add,
                        )

            return [lambda: half(0, 4), lambda: half(4, D // 128)]

        def v_group(lb):
            # V_sb[l, h*64+hs] = (X @ W_v + bias) * mask_l, both pairs as two
            # interleaved chains (shared xT lhsT, alternating PSUM banks).
            # Returns two emission chunks.
            state = {}

            def half(lo, hi):
                if lo == 0:
                    state["psva"] = psA.tile([128, 512], F32, tag="psA", name="psva")
                    state["psvb"] = psA.tile([128, 512], F32, tag="psA", name="psvb")
                psva, psvb = state["psva"], state["psvb"]
                for dc in range(lo, hi):
                    xc = xw_r[:, dc, lb * 128:(lb + 1) * 128]
                    nc.tensor.matmul(
                        psva[:, 0:128], xc, xw_r[:, dc, L + 512:L + 640],
                        start=(dc == 0), stop=False,
                    )
                    nc.tensor.matmul(
                        psvb[:, 0:128], xc, xw_r[:, dc, L + 640:L + 768],
                        start=(dc == 0), stop=False,
                    )
                if hi == D // 128:
                    nc.tensor.matmul(
                        psva[:, 0:128], ones1[:, :], biasv_sb[:, 0:128],
                        start=False, stop=True,
                    )
                    nc.tensor.matmul(
                        psvb[:, 0:128], ones1[:, :], biasv_sb[:, 128:256],
                        start=False, stop=True,
                    )
                    for pr, psv in ((0, psva), (1, psvb)):
                        nc.vector.tensor_scalar(
                            vsb[:, lb, 2 * pr:2 * pr + 2, 0:64],
                            psv[:, 0:128].rearrange("p (h c) -> p h c", h=2),
                            mask_sb[:, lb:lb + 1], None, Alu.mult,
                        )

            return [lambda: half(0, 4), lambda: half(4, D // 128)]

        def q_ap(h, c0, c1):
            p0 = (h % 2) * 64
            return qkvT[p0:p0 + 64, h // 2, c0:c1]

        def k_ap(h, c0, c1):
            p0 = (h % 2) * 64
            return qkvT[p0:p0 + 64, 2 + h // 2, c0:c1]

        def attention(pr, fillers=None):
            # One head-pair. pr==0: ALiBi pair - O accumulation split into
            # phases (slot 0=below diag, 1=crossing, 2=above); non-cross
            # tiles multiply by shared Ea (broadcast AP) and fold the
            # per-partition alibi row factor into V; crossing tiles use the
            # streamed per-head Ecross. pr==1: flat pair, single phase
            # (slot 1), plain Ea multiply, V unscaled.
            # fillers[(qh, kb)] = phase-A emitters interleaved into the
            # stream right after that iteration.
            he, ho = 2 * pr, 2 * pr + 1
            for qh in range(NQH):
                q0 = qh * QW
                if pr == 0:
                    phases = []
                    if qh > 0:
                        phases.append((0, list(range(0, 4 * qh))))
                    phases.append((1, list(range(4 * qh, 4 * qh + 4))))
                    if qh < NQH - 1:
                        phases.append((2, list(range(4 * qh + 4, NKB))))
                else:
                    phases = [(1, list(range(NKB)))]
                for slot, kbs in phases:
                    ope = psO.tile([65, QW], F32, tag="ope", name="ope")
                    opo = psO.tile([65, QW], F32, tag="opo", name="opo")
                    for i, kb in enumerate(kbs):
                        first, last = (i == 0), (i == len(kbs) - 1)
                        ps_s = psS.tile([128, 2 * QW], F32, tag="ps_s")
                        nc.tensor.matmul(
                            ps_s[:, 0:QW],
                            k_ap(he, kb * 128, (kb + 1) * 128),
                            q_ap(he, q0, q0 + QW), start=True, stop=True,
                        )
                        nc.tensor.matmul(
                            ps_s[:, QW:2 * QW],
                            k_ap(ho, kb * 128, (kb + 1) * 128),
                            q_ap(ho, q0, q0 + QW), start=True, stop=True,
                        )
                        pT = pp.tile([128, 2 * QW], BF16, tag="pT")
                        nc.scalar.activation(pT[:], ps_s[:], Act.Exp)
                        pb = pq.tile([128, 2 * QW], BF16, tag="pb")
                        cross = (pr == 0 and slot == 1)
                        if cross:
                            j = kb - 4 * qh
                            ec = pe.tile([128, 2 * QW], BF16, tag="ec")
                            nc.sync.dma_start(ec[:], ecross_d[qh, j])
                            nc.vector.tensor_tensor(
                                pb[:], pT[:], ec[:], Alu.mult)
                            lhs_e = vsb[:, kb, he, 0:65]
                            lhs_o = vsb[:, kb, ho, 0:65]
                        else:
                            ea_b = ea[:, kb, None, q0:q0 + QW].broadcast_to(
                                [128, 2, QW])
                            nc.vector.tensor_tensor(
                                pb[:].rearrange("p (j q) -> p j q", j=2),
                                pT[:].rearrange("p (j q) -> p j q", j=2),
                                ea_b, Alu.mult)
                            if pr == 0:
                                col = (qh * NKB + kb) * 2
                                vt = vp.tile([128, 2, 65], BF16, tag="vt")
                                nc.vector.tensor_scalar(
                                    vt[:, 0, :], vsb[:, kb, he, 0:65],
                                    rowfac_sb[:, col:col + 1], None, Alu.mult)
                                nc.vector.tensor_scalar(
                                    vt[:, 1, :], vsb[:, kb, ho, 0:65],
                                    rowfac_sb[:, col + 1:col + 2], None,
                                    Alu.mult)
                                lhs_e = vt[:, 0, :]
                                lhs_o = vt[:, 1, :]
                            else:
                                lhs_e = vsb[:, kb, he, 0:65]
                                lhs_o = vsb[:, kb, ho, 0:65]
                        nc.tensor.matmul(
                            ope[:], lhs_e, pb[:, 0:QW],
                            start=first, stop=last,
                        )
                        nc.tensor.matmul(
                            opo[:], lhs_o, pb[:, QW:2 * QW],
                            start=first, stop=last,
                        )
                        if fillers:
                            # stamp fillers with a virtual not-before time
                            # near their slot so the scheduler doesn't flood
                            # the PE FIFO with them ahead of the critical
                            # S->exp chain
                            g = qh * NKB + kb
                            est_ms = (28.0 + 1.5 * g) * 1e-3
                            for fn in fillers.get((qh, kb), ()):
                                with tc.tile_wait_until(est_ms):
                                    fn()
                    for hh, op_t in ((he, ope), (ho, opo)):
                        ot = outp.tile([65, QW], F32, tag="ot")
                        nc.vector.tensor_copy(ot[:], op_t[:])
                        nc.sync.dma_start(
                            oun_d[hh, slot, :, q0:q0 + QW], ot[:])

        # Emission order is program order (producers must precede their
        # consumers), but WITHIN phase A we order pair 0's groups by when
        # attention(0) first needs them, so the scheduler can start
        # attention(0)'s pipeline (and the ACT engine) as early as possible
        # while the rest of phase A fills PE idle time.
        for c in t_group(2, 0, 1):   # K pair0, kb 0-7
            c()
        for c in t_group(0, 0, 1):   # Q pair0, qh 0-1
            c()
        for c in v_group(0):
            c()
        fillers = {}

        def put(qh, kb, chunk):
            fillers.setdefault((qh, kb), []).append(chunk)

        for c in v_group(1):         # consumed at kb == 1
            put(0, 0, c)
        c1, c2 = v_group(2)          # consumed at kb == 2
        put(0, 0, c1)
        put(0, 1, c2)
        for lb in range(3, NKB):     # V block lb consumed at kb == lb
            c1, c2 = v_group(lb)
            put(0, lb - 3, c1)
            put(0, lb - 2, c2)
        for i, c in enumerate(t_group(2, 2, 3)):   # K pair0 kb 8-15
            put(0, 4 + i, c)
        for i, c in enumerate(t_group(0, 2, 3)):   # Q pair0 qh 2-3
            put(0, 13 + i, c)
        for i, c in enumerate(t_group(3, 0, 1)):   # K pair1 kb 0-7
            put(1, 1 + i, c)
        for i, c in enumerate(t_group(1, 0, 1)):   # Q pair1 qh 0-1
            put(1, 8 + i, c)
        for i, c in enumerate(t_group(3, 2, 3)):   # K pair1 kb 8-15
            put(2, 1 + i, c)
        for i, c in enumerate(t_group(1, 2, 3)):   # Q pair1 qh 2-3
            put(2, 8 + i, c)
        attention(0, fillers)
        attention(1)

    nc.compile()
    return nc


def _prep_inputs(x, adj, mask, weights, in_bias):
    import ml_dtypes
    bf16 = ml_dtypes.bfloat16

    wq = np.array(weights, dtype=np.float32, copy=True)
    bq = np.array(in_bias, dtype=np.float32, copy=True).reshape(3 * D)
    for h in range(NH):
        wq[:, h * 192:h * 192 + 64] *= 0.125
        bq[h * 192:h * 192 + 64] *= 0.125

    in_maps = []
    for c in range(8):
        b = c // HPC
        heads = _core_heads(c)
        # QK cols: [Q_h0..Q_h3 | K_h0..K_h3], V cols: [V_h0..V_h3]
        perm_qk = np.concatenate([
            np.arange(H * 192 + which * 64, H * 192 + which * 64 + 64)
            for which in range(2) for H in heads
        ])
        perm_v = np.concatenate([
            np.arange(H * 192 + 128, H * 192 + 192) for H in heads
        ])
        xw = np.ascontiguousarray(np.concatenate(
            [x[b].T, wq[:, perm_qk], wq[:, perm_v]], axis=1)).astype(bf16)
        biasqk = np.ascontiguousarray(bq[perm_qk].reshape(4, 128).T)
        biasv = np.ascontiguousarray(bq[perm_v].reshape(1, 256)).astype(bf16)
        maskf = mask[b].astype(np.float32)
        mask16 = np.ascontiguousarray(maskf.reshape(NKB, 128).T)
        in_maps.append({
            "xw": xw, "biasqk": biasqk,
            "biasv": biasv, "mask16": mask16,
            "ea": None, "ecross": None, "rowfac": None,  # filled in kernel()
            "_b": b, "_heads": heads,
        })
    return in_maps


def _reference_numpy(x, adj, mask, weights, in_bias, out_bias, gamma):
    # correct fallback for inputs the fast path doesn't cover
    slopes = _alibi_slopes_full()
    pos = np.arange(L, dtype=np.float32)
    rel = -np.abs(pos[None, :] - pos[:, None])
    out = np.empty((B, L, D), dtype=np.float32)
    qkv = x @ weights + in_bias.reshape(1, 1, 3 * D)
    gamma = gamma.reshape(NH)
    for b in range(B):
        for h in range(NH):
            q = qkv[b, :, h * 192:h * 192 + 64]
            k = qkv[b, :, h * 192 + 64:h * 192 + 128]
            v = qkv[b, :, h * 192 + 128:h * 192 + 192]
            s = q @ k.T / 8.0 + slopes[h] * rel + gamma[h] * adj[b, 0]
            s = s - s.max(axis=1, keepdims=True)
            p = np.exp(s)
            p /= p.sum(axis=1, keepdims=True)
            m2 = (mask[b][:, None] & mask[b][None, :]).astype(np.float32)
            out[b, :, h * 64:(h + 1) * 64] = (p * m2) @ v
    return out + out_bias.reshape(1, 1, D)


def kernel(x, adj, mask, weights, in_bias, out_bias, gamma):
    import os
    import ml_dtypes
    from concourse.bass_utils import run_bass_kernel_spmd

    bf16 = ml_dtypes.bfloat16

    x = np.asarray(x, dtype=np.float32)
    adj = np.asarray(adj, dtype=np.float32)
    mask_np = np.asarray(mask)
    weights = np.asarray(weights, dtype=np.float32)
    in_bias = np.asarray(in_bias, dtype=np.float32)
    out_bias = np.asarray(out_bias, dtype=np.float32)
    gamma_np = np.asarray(gamma, dtype=np.float32).reshape(NH)
    slopes_full = _alibi_slopes_full()

    if not np.all(gamma_np == gamma_np[0]):
        # shared-Ea fast path needs uniform gamma; fall back to exact host
        return _reference_numpy(
            x, adj, mask_np, weights, in_bias, out_bias,
            np.asarray(gamma, dtype=np.float32))
    g0 = float(gamma_np[0])

    if "nc" not in _cache:
        _cache["nc"] = _build()
    nc = _cache["nc"]
    trace = os.environ.get("BASS_TRACE", "0") == "1"

    in_maps = _prep_inputs(x, adj, mask_np, weights, in_bias)

    kidx = np.arange(L, dtype=np.float32)
    # per-batch shared Ea (fp32 host, bf16 device)
    ea_by_b = []
    for b in range(B):
        adjT = adj[b, 0].T
        ea_f = np.exp(g0 * adjT).astype(np.float32)  # [k, q]
        ea_by_b.append(ea_f)

    for c, m in enumerate(in_maps):
        b, heads = m.pop("_b"), m.pop("_heads")
        ea_f = ea_by_b[b]
        # device layout [p, kb, q] -> flat [128, NKB*L]
        ea_dev = np.ascontiguousarray(
            ea_f.reshape(NKB, 128, L).transpose(1, 0, 2).reshape(128, NKB * L)
        ).astype(bf16)
        m["ea"] = ea_dev

        # Ecross[qh, j, p, hh*QW+ql] for the ALiBi pair (local heads 0,1)
        s0, s1 = slopes_full[heads[0]], slopes_full[heads[1]]
        ecross = np.empty((NQH, 4, 128, 2 * QW), dtype=bf16)
        for qh in range(NQH):
            q_idx = kidx[qh * QW:(qh + 1) * QW]
            for j in range(4):
                kb = 4 * qh + j
                k_idx = kidx[kb * 128:(kb + 1) * 128]
                absd = np.abs(k_idx[:, None] - q_idx[None, :])
                base = ea_f[kb * 128:(kb + 1) * 128, qh * QW:(qh + 1) * QW]
                ecross[qh, j, :, 0:QW] = (base * np.exp(-s0 * absd))
                ecross[qh, j, :, QW:] = (base * np.exp(-s1 * absd))
        m["ecross"] = ecross

        # rowfac[p, ((qh*NKB+kb)*2 + hh)] fp32
        rowfac = np.ones((128, NQH, NKB, 2), dtype=np.float32)
        for qh in range(NQH):
            q0 = qh * QW
            for kb in range(NKB):
                if 4 * qh <= kb < 4 * qh + 4:
                    continue
                k_idx = kidx[kb * 128:(kb + 1) * 128]
                for hh, s in ((0, s0), (1, s1)):
                    if kb < 4 * qh:      # below diag: k < q0
                        rowfac[:, qh, kb, hh] = np.exp(s * (k_idx - q0))
                    else:                # above diag: k >= q0+512
                        rowfac[:, qh, kb, hh] = np.exp(-s * (k_idx - q0 - 511))
        m["rowfac"] = np.ascontiguousarray(rowfac.reshape(128, -1))

    res = run_bass_kernel_spmd(nc, in_maps, list(range(8)), trace=trace)
    _cache["last_res"] = res

    ql = np.arange(QW, dtype=np.float32)
    out = np.empty((B, L, D), dtype=np.float32)
    for c in range(8):
        b = c // HPC
        heads = _core_heads(c)
        oun = res.results[c]["o_un"]  # [HPC, 3, 65, L]
        maskf = mask_np[b].astype(np.float32)
        for hl, Hg in enumerate(heads):
            s = slopes_full[Hg]
            acc = np.empty((65, L), dtype=np.float32)
            for qh in range(NQH):
                sl = slice(qh * QW, (qh + 1) * QW)
                if hl < 2:
                    o_q = oun[hl, 1, :, sl].copy()
                    if qh > 0:
                        o_q += oun[hl, 0, :, sl] * np.exp(-s * ql)[None, :]
                    if qh < NQH - 1:
                        o_q += oun[hl, 2, :, sl] * \
                            np.exp(s * (ql - (QW - 1)))[None, :]
                else:
                    o_q = oun[hl, 1, :, sl]
                acc[:, sl] = o_q
            denom = acc[64, :]
            o_h = (acc[:64, :] / denom[None, :]) * maskf[None, :]
            out[b, :, Hg * HS:(Hg + 1) * HS] = o_h.T
    out += out_bias.reshape(1, 1, D)
    return out


# revision 3
# speedup vs baseline: 1.0193x; 1.0193x over previous
"""Trainium2 Bass kernel for MultiHeadSelfAttention with ALiBi + adjacency bias.

Sharding: 8 cores = 2 batches x 4 pair-groups. Core c (b=c//4, a=c%4) owns
heads [2a, 2a+1, 8+2a, 9+2a]: pair0 = ALiBi heads (slopes 2^-(h+1)),
pair1 = flat heads (slope 0).

Design (all matmuls bf16):
  A) qkvT[c, l] = (W_qk^T @ X^T) (transposed, head-major cols, 1/8 folded
     into Q); V_sb[l, h, hs] = X @ W_v (+bias), masked by mask_k, plus a
     ones column per head -> V_aug lhsT [k, 65].
  B) Shared Ea = exp(gamma*adjT) bf16 [2048, 2048] SBUF-RESIDENT (8.4MB,
     loaded once) - replaces the per-head E DMA (was 33.5MB/core).
     ALiBi factor exp(-s|k-q|) decomposes per (qh, kb) tile:
       below-diag (k < q0):        exp(s(k-q0))     * exp(-s(q-q0))
       above-diag (k >= q0+512):   exp(-s(k-q0-511))* exp(s(q-q0-511))
     row part (per-partition k) -> folded into V via tensor_scalar [128,65];
     col part (per-q) -> applied on HOST: O accumulated in 3 PSUM phases
     (below/cross/above), each drained separately; host combines.
     Diagonal-crossing tiles use host-precomputed Ecross = Ea*exp(-s|k-q|)
     (bf16, streamed, 4.2MB/core).
  C) per head-pair, per (qh, kb): S^T[k,q] = K Q^T/8 in PSUM fp32
     (concurrent PE row tiles 0-63/64-127), pT = exp(S^T) on ACT
     (PSUM->SBUF bf16, one op for both heads), pb = pT * Ea (DVE bf16,
     broadcast AP reads the 512-wide Ea tile twice), O^T_aug[65,q] +=
     V_aug^T @ pb accumulated per phase; row 64 = denom.
  Host: combine phase partials with col factors, divide by denom, apply
  mask_q, transpose per-head, assemble, +out_bias.
"""

import math

import numpy as np

B, L, D = 2, 2048, 1024
NH, HS = 16, 64
HPC = 4          # heads per core
NKB = L // 128   # 16 k blocks
QW = 512         # q tile width (1 PSUM bank)
NQH = L // QW    # 4 q tiles

_cache = {}


def _alibi_slopes_full():
    ah = NH // 2
    start = 2.0 ** (-(2.0 ** -(math.log2(ah) - 3)))
    s = [start * (start ** i) for i in range(ah)]
    return np.array(s + [0.0] * (NH - ah), dtype=np.float32)


def _core_heads(c):
    a = c % HPC
    return [2 * a, 2 * a + 1, 8 + 2 * a, 9 + 2 * a]


def _build():
    import concourse.tile as tile
    import concourse.mybir as mybir
    from concourse import bacc
    from contextlib import ExitStack

    dt = mybir.dt
    F32, BF16 = dt.float32, dt.bfloat16
    Alu = mybir.AluOpType
    Act = mybir.ActivationFunctionType

    nc = bacc.Bacc("TRN2", target_bir_lowering=False, num_devices=8)

    # xT | wqk | wv concatenated: one DMA per 128-row chunk of D
    xw_d = nc.dram_tensor("xw", [D, L + 512 + 256], BF16, kind="ExternalInput")
    biasqk_d = nc.dram_tensor("biasqk", [128, 4], F32, kind="ExternalInput")
    biasv_d = nc.dram_tensor("biasv", [1, 256], BF16, kind="ExternalInput")
    mask16_d = nc.dram_tensor("mask16", [128, NKB], F32, kind="ExternalInput")
    ea_d = nc.dram_tensor("ea", [128, NKB * L], BF16, kind="ExternalInput")
    ecross_d = nc.dram_tensor(
        "ecross", [NQH, 4, 128, 2 * QW], BF16, kind="ExternalInput")
    rowfac_d = nc.dram_tensor(
        "rowfac", [128, NQH * NKB * 2], F32, kind="ExternalInput")
    oun_d = nc.dram_tensor("o_un", [HPC, 3, 65, L], F32, kind="ExternalOutput")

    with tile.TileContext(nc) as tc, ExitStack() as ctx:
        persist = ctx.enter_context(tc.tile_pool(name="persist", bufs=1))
        # Q^T,K^T bf16: mb 0-1 = Q pairs (h on part 0-63/64-127), 2-3 = K
        qkvT = persist.tile([128, 4, L], BF16)
        # V_aug: [k_part, kb, h, 66] - cols 0:64 = V*mask, col 64 = ones
        vsb = persist.tile([128, NKB, HPC, 66], BF16)
        # shared exp(gamma*adjT): [k_part, kb, q]
        ea = persist.tile([128, NKB, L], BF16)
        rowfac_sb = persist.tile([128, NQH * NKB * 2], F32)

        pa = ctx.enter_context(tc.tile_pool(name="phaseA", bufs=1))
        pe = ctx.enter_context(tc.tile_pool(name="pe", bufs=4))
        pp = ctx.enter_context(tc.tile_pool(name="pp", bufs=8))
        pq = ctx.enter_context(tc.tile_pool(name="pq", bufs=8))
        vp = ctx.enter_context(tc.tile_pool(name="vp", bufs=4))
        outp = ctx.enter_context(tc.tile_pool(name="outp", bufs=4))
        psA = ctx.enter_context(tc.tile_pool(name="psA", bufs=2, space="PSUM"))
        psS = ctx.enter_context(tc.tile_pool(name="psS", bufs=2, space="PSUM"))
        psO = ctx.enter_context(tc.tile_pool(name="psO", bufs=1, space="PSUM"))

        xw_r = pa.tile([128, D // 128, L + 512 + 256], BF16)
        xw_dv = xw_d.rearrange("(o p) c -> p o c", p=128)
        for kc in range(D // 128):
            nc.sync.dma_start(xw_r[:, kc, :], xw_dv[:, kc, :])
        biasqk_sb = pa.tile([128, 4], F32)
        nc.sync.dma_start(biasqk_sb[:], biasqk_d[:])
        biasv_sb = pa.tile([1, 256], BF16)
        nc.sync.dma_start(biasv_sb[:], biasv_d[:])
        mask_sb = pa.tile([128, NKB], F32)
        nc.sync.dma_start(mask_sb[:], mask16_d[:])
        nc.sync.dma_start(rowfac_sb[:], rowfac_d[:])
        # resident Ea: kb 4..15 first (first non-cross consumers), then 0..3
        for kb in list(range(4, NKB)) + list(range(4)):
            nc.sync.dma_start(ea[:, kb, :], ea_d[:, kb * L:(kb + 1) * L])
        ones1 = pa.tile([1, 128], BF16)
        nc.vector.memset(ones1[:], 1.0)
        nc.vector.memset(vsb[:, :, :, 64:65], 1.0)
        # tiny dummy exp: pulls the ~2.7us ACT_TABLE_LOAD into the DMA ramp
        wtmp = pa.tile([1, 16], F32)
        nc.vector.memset(wtmp[:], 0.0)
        wex = pa.tile([1, 16], BF16)
        nc.scalar.activation(wex[:], wtmp[:], Act.Exp)

        def t_group(mb, nqa, nqb):
            # qkvT[c, l] blocks (mb, nqa) and (mb, nqb) via two interleaved
            # accumulation chains on different PSUM banks sharing each
            # LDWEIGHTS -> fills/drains overlap instead of serializing.
            # Returns two emission chunks (for fine-grained interleaving).
            state = {}

            def half(lo, hi):
                if lo == 0:
                    state["psa"] = psA.tile([128, 512], F32, tag="psA", name="psa")
                    state["psb"] = psA.tile([128, 512], F32, tag="psA", name="psb")
                psa, psb = state["psa"], state["psb"]
                for kc in range(lo, hi):
                    w = xw_r[:, kc, L + mb * 128:L + (mb + 1) * 128]
                    nc.tensor.matmul(
                        psa[:], w, xw_r[:, kc, nqa * 512:(nqa + 1) * 512],
                        start=(kc == 0), stop=(kc == D // 128 - 1),
                    )
                    nc.tensor.matmul(
                        psb[:], w, xw_r[:, kc, nqb * 512:(nqb + 1) * 512],
                        start=(kc == 0), stop=(kc == D // 128 - 1),
                    )
                if hi == D // 128:
                    for nq, ps in ((nqa, psa), (nqb, psb)):
                        nc.vector.tensor_scalar(
                            qkvT[:, mb, nq * 512:(nq + 1) * 512], ps[:],
                            biasqk_sb[:, mb:mb + 1], None, Alu.add,
                        )

            return [lambda: half(0, 4), lambda: half(4, D // 128)]

        def v_group(lb):
            # V_sb[l, h*64+hs] = (X @ W_v + bias) * mask_l, both pairs as two
            # interleaved chains (shared xT lhsT, alternating PSUM banks).
            # Returns two emission chunks.
            state = {}

            def half(lo, hi):
                if lo == 0:
                    state["psva"] = psA.tile([128, 512], F32, tag="psA", name="psva")
                    state["psvb"] = psA.tile([128, 512], F32, tag="psA", name="psvb")
                psva, psvb = state["psva"], state["psvb"]
                for dc in range(lo, hi):
                    xc = xw_r[:, dc, lb * 128:(lb + 1) * 128]
                    nc.tensor.matmul(
                        psva[:, 0:128], xc, xw_r[:, dc, L + 512:L + 640],
                        start=(dc == 0), stop=False,
                    )
                    nc.tensor.matmul(
                        psvb[:, 0:128], xc, xw_r[:, dc, L + 640:L + 768],
                        start=(dc == 0), stop=False,
                    )
                if hi == D // 128:
                    nc.tensor.matmul(
                        psva[:, 0:128], ones1[:, :], biasv_sb[:, 0:128],
                        start=False, stop=True,
                    )
                    nc.tensor.matmul(
                        psvb[:, 0:128], ones1[:, :], biasv_sb[:, 128:256],
                        start=False, stop=True,
                    )
                    for pr, psv in ((0, psva), (1, psvb)):
                        nc.vector.tensor_scalar(
                            vsb[:, lb, 2 * pr:2 * pr + 2, 0:64],
                            psv[:, 0:128].rearrange("p (h c) -> p h c", h=2),
                            mask_sb[:, lb:lb + 1], None, Alu.mult,
                        )

            return [lambda: half(0, 4), lambda: half(4, D // 128)]

        def q_ap(h, c0, c1):
            p0 = (h % 2) * 64
            return qkvT[p0:p0 + 64, h // 2, c0:c1]

        def k_ap(h, c0, c1):
            p0 = (h % 2) * 64
            return qkvT[p0:p0 + 64, 2 + h // 2, c0:c1]

        def attention(pr, fillers=None):
            # One head-pair. pr==0: ALiBi pair - O accumulation split into
            # phases (slot 0=below diag, 1=crossing, 2=above); non-cross
            # tiles multiply by shared Ea (broadcast AP) and fold the
            # per-partition alibi row factor into V; crossing tiles use the
            # streamed per-head Ecross. pr==1: flat pair, single phase
            # (slot 1), plain Ea multiply, V unscaled.
            # Software-pipelined: each iteration's S-pair is EMITTED before
            # the previous iteration's exp/mult/O tail, so in the PE's
            # strict-FIFO queue S(k+1) sits ahead of O(k) (which waits on
            # DVE) - the PE computes next scores instead of head-of-line
            # blocking, and the ACT engine stays fed.
            # fillers[(qh, kb)] = phase-A emitters interleaved into the
            # stream right after that iteration.
            he, ho = 2 * pr, 2 * pr + 1

            def emit_tail(st):
                (qh, slot, kb, ps_s, ope, opo, first, last) = st
                q0 = qh * QW
                pT = pp.tile([128, 2 * QW], BF16, tag="pT")
                nc.scalar.activation(pT[:], ps_s[:], Act.Exp)
                cross = (pr == 0 and slot == 1)
                if pr == 0 and not cross:
                    col = (qh * NKB + kb) * 2
                    vt = vp.tile([128, 2, 65], BF16, tag="vt")
                    nc.vector.tensor_scalar(
                        vt[:, 0, :], vsb[:, kb, he, 0:65],
                        rowfac_sb[:, col:col + 1], None, Alu.mult)
                    nc.vector.tensor_scalar(
                        vt[:, 1, :], vsb[:, kb, ho, 0:65],
                        rowfac_sb[:, col + 1:col + 2], None, Alu.mult)
                    lhs_e, lhs_o = vt[:, 0, :], vt[:, 1, :]
                else:
                    lhs_e = vsb[:, kb, he, 0:65]
                    lhs_o = vsb[:, kb, ho, 0:65]
                pb = pq.tile([128, 2 * QW], BF16, tag="pb")
                if cross:
                    j = kb - 4 * qh
                    ec = pe.tile([128, 2 * QW], BF16, tag="ec")
                    nc.sync.dma_start(ec[:], ecross_d[qh, j])
                    nc.vector.tensor_tensor(pb[:], pT[:], ec[:], Alu.mult)
                else:
                    ea_b = ea[:, kb, None, q0:q0 + QW].broadcast_to(
                        [128, 2, QW])
                    nc.vector.tensor_tensor(
                        pb[:].rearrange("p (j q) -> p j q", j=2),
                        pT[:].rearrange("p (j q) -> p j q", j=2),
                        ea_b, Alu.mult)
                nc.tensor.matmul(
                    ope[:], lhs_e, pb[:, 0:QW], start=first, stop=last)
                nc.tensor.matmul(
                    opo[:], lhs_o, pb[:, QW:2 * QW], start=first, stop=last)
                if last:
                    for hh, op_t in ((he, ope), (ho, opo)):
                        ot = outp.tile([65, QW], F32, tag="ot")
                        nc.vector.tensor_copy(ot[:], op_t[:])
                        nc.sync.dma_start(
                            oun_d[hh, slot, :, q0:q0 + QW], ot[:])
                if fillers:
                    # stamp fillers with a virtual not-before time near
                    # their slot so the scheduler doesn't flood the PE
                    # FIFO with them ahead of the critical S->exp chain
                    g = qh * NKB + kb
                    est_ms = (28.0 + 1.5 * g) * 1e-3
                    for fn in fillers.get((qh, kb), ()):
                        with tc.tile_wait_until(est_ms):
                            fn()

            pending = None
            for qh in range(NQH):
                q0 = qh * QW
                if pr == 0:
                    phases = []
                    if qh > 0:
                        phases.append((0, list(range(0, 4 * qh))))
                    phases.append((1, list(range(4 * qh, 4 * qh + 4))))
                    if qh < NQH - 1:
                        phases.append((2, list(range(4 * qh + 4, NKB))))
                else:
                    phases = [(1, list(range(NKB)))]
                for slot, kbs in phases:
                    ope = psO.tile([65, QW], F32, tag="ope", name="ope")
                    opo = psO.tile([65, QW], F32, tag="opo", name="opo")
                    for i, kb in enumerate(kbs):
                        first, last = (i == 0), (i == len(kbs) - 1)
                        ps_s = psS.tile([128, 2 * QW], F32, tag="ps_s")
                        nc.tensor.matmul(
                            ps_s[:, 0:QW],
                            k_ap(he, kb * 128, (kb + 1) * 128),
                            q_ap(he, q0, q0 + QW), start=True, stop=True,
                        )
                        nc.tensor.matmul(
                            ps_s[:, QW:2 * QW],
                            k_ap(ho, kb * 128, (kb + 1) * 128),
                            q_ap(ho, q0, q0 + QW), start=True, stop=True,
                        )
                        if pending is not None:
                            emit_tail(pending)
                        pending = (qh, slot, kb, ps_s, ope, opo, first, last)
            emit_tail(pending)

        # Emission order is program order (producers must precede their
        # consumers), but WITHIN phase A we order pair 0's groups by when
        # attention(0) first needs them, so the scheduler can start
        # attention(0)'s pipeline (and the ACT engine) as early as possible
        # while the rest of phase A fills PE idle time.
        for c in t_group(2, 0, 1):   # K pair0, kb 0-7
            c()
        for c in t_group(0, 0, 1):   # Q pair0, qh 0-1
            c()
        for c in v_group(0):
            c()
        fillers = {}

        def put(qh, kb, chunk):
            fillers.setdefault((qh, kb), []).append(chunk)

        for c in v_group(1):         # consumed at kb == 1
            put(0, 0, c)
        c1, c2 = v_group(2)          # consumed at kb == 2
        put(0, 0, c1)
        put(0, 1, c2)
        for lb in range(3, NKB):     # V block lb consumed at kb == lb
            c1, c2 = v_group(lb)
            put(0, lb - 3, c1)
            put(0, lb - 2, c2)
        for i, c in enumerate(t_group(2, 2, 3)):   # K pair0 kb 8-15
            put(0, 4 + i, c)
        for i, c in enumerate(t_group(0, 2, 3)):   # Q pair0 qh 2-3
            put(0, 13 + i, c)
        for i, c in enumerate(t_group(3, 0, 1)):   # K pair1 kb 0-7
            put(1, 1 + i, c)
        for i, c in enumerate(t_group(1, 0, 1)):   # Q pair1 qh 0-1
            put(1, 8 + i, c)
        for i, c in enumerate(t_group(3, 2, 3)):   # K pair1 kb 8-15
            put(2, 1 + i, c)
        for i, c in enumerate(t_group(1, 2, 3)):   # Q pair1 qh 2-3
            put(2, 8 + i, c)
        attention(0, fillers)
        attention(1)

    nc.compile()
    return nc


def _prep_inputs(x, adj, mask, weights, in_bias):
    import ml_dtypes
    bf16 = ml_dtypes.bfloat16

    wq = np.array(weights, dtype=np.float32, copy=True)
    bq = np.array(in_bias, dtype=np.float32, copy=True).reshape(3 * D)
    for h in range(NH):
        wq[:, h * 192:h * 192 + 64] *= 0.125
        bq[h * 192:h * 192 + 64] *= 0.125

    in_maps = []
    for c in range(8):
        b = c // HPC
        heads = _core_heads(c)
        # QK cols: [Q_h0..Q_h3 | K_h0..K_h3], V cols: [V_h0..V_h3]
        perm_qk = np.concatenate([
            np.arange(H * 192 + which * 64, H * 192 + which * 64 + 64)
            for which in range(2) for H in heads
        ])
        perm_v = np.concatenate([
            np.arange(H * 192 + 128, H * 192 + 192) for H in heads
        ])
        xw = np.ascontiguousarray(np.concatenate(
            [x[b].T, wq[:, perm_qk], wq[:, perm_v]], axis=1)).astype(bf16)
        biasqk = np.ascontiguousarray(bq[perm_qk].reshape(4, 128).T)
        biasv = np.ascontiguousarray(bq[perm_v].reshape(1, 256)).astype(bf16)
        maskf = mask[b].astype(np.float32)
        mask16 = np.ascontiguousarray(maskf.reshape(NKB, 128).T)
        in_maps.append({
            "xw": xw, "biasqk": biasqk,
            "biasv": biasv, "mask16": mask16,
            "ea": None, "ecross": None, "rowfac": None,  # filled in kernel()
            "_b": b, "_heads": heads,
        })
    return in_maps


def _reference_numpy(x, adj, mask, weights, in_bias, out_bias, gamma):
    # correct fallback for inputs the fast path doesn't cover
    slopes = _alibi_slopes_full()
    pos = np.arange(L, dtype=np.float32)
    rel = -np.abs(pos[None, :] - pos[:, None])
    out = np.empty((B, L, D), dtype=np.float32)
    qkv = x @ weights + in_bias.reshape(1, 1, 3 * D)
    gamma = gamma.reshape(NH)
    for b in range(B):
        for h in range(NH):
            q = qkv[b, :, h * 192:h * 192 + 64]
            k = qkv[b, :, h * 192 + 64:h * 192 + 128]
            v = qkv[b, :, h * 192 + 128:h * 192 + 192]
            s = q @ k.T / 8.0 + slopes[h] * rel + gamma[h] * adj[b, 0]
            s = s - s.max(axis=1, keepdims=True)
            p = np.exp(s)
            p /= p.sum(axis=1, keepdims=True)
            m2 = (mask[b][:, None] & mask[b][None, :]).astype(np.float32)
            out[b, :, h * 64:(h + 1) * 64] = (p * m2) @ v
    return out + out_bias.reshape(1, 1, D)


def kernel(x, adj, mask, weights, in_bias, out_bias, gamma):
    import os
    import ml_dtypes
    from concourse.bass_utils import run_bass_kernel_spmd

    bf16 = ml_dtypes.bfloat16

    x = np.asarray(x, dtype=np.float32)
    adj = np.asarray(adj, dtype=np.float32)
    mask_np = np.asarray(mask)
    weights = np.asarray(weights, dtype=np.float32)
    in_bias = np.asarray(in_bias, dtype=np.float32)
    out_bias = np.asarray(out_bias, dtype=np.float32)
    gamma_np = np.asarray(gamma, dtype=np.float32).reshape(NH)
    slopes_full = _alibi_slopes_full()

    if not np.all(gamma_np == gamma_np[0]):
        # shared-Ea fast path needs uniform gamma; fall back to exact host
        return _reference_numpy(
            x, adj, mask_np, weights, in_bias, out_bias,
            np.asarray(gamma, dtype=np.float32))
    g0 = float(gamma_np[0])

    if "nc" not in _cache:
        _cache["nc"] = _build()
    nc = _cache["nc"]
    trace = os.environ.get("BASS_TRACE", "0") == "1"

    in_maps = _prep_inputs(x, adj, mask_np, weights, in_bias)

    kidx = np.arange(L, dtype=np.float32)
    # per-batch shared Ea (fp32 host, bf16 device)
    ea_by_b = []
    for b in range(B):
        adjT = adj[b, 0].T
        ea_f = np.exp(g0 * adjT).astype(np.float32)  # [k, q]
        ea_by_b.append(ea_f)

    for c, m in enumerate(in_maps):
        b, heads = m.pop("_b"), m.pop("_heads")
        ea_f = ea_by_b[b]
        # device layout [p, kb, q] -> flat [128, NKB*L]
        ea_dev = np.ascontiguousarray(
            ea_f.reshape(NKB, 128, L).transpose(1, 0, 2).reshape(128, NKB * L)
        ).astype(bf16)
        m["ea"] = ea_dev

        # Ecross[qh, j, p, hh*QW+ql] for the ALiBi pair (local heads 0,1)
        s0, s1 = slopes_full[heads[0]], slopes_full[heads[1]]
        ecross = np.empty((NQH, 4, 128, 2 * QW), dtype=bf16)
        for qh in range(NQH):
            q_idx = kidx[qh * QW:(qh + 1) * QW]
            for j in range(4):
                kb = 4 * qh + j
                k_idx = kidx[kb * 128:(kb + 1) * 128]
                absd = np.abs(k_idx[:, None] - q_idx[None, :])
                base = ea_f[kb * 128:(kb + 1) * 128, qh * QW:(qh + 1) * QW]
                ecross[qh, j, :, 0:QW] = (base * np.exp(-s0 * absd))
                ecross[qh, j, :, QW:] = (base * np.exp(-s1 * absd))
        m["ecross"] = ecross

        # rowfac[p, ((qh*NKB+kb)*2 + hh)] fp32
        rowfac = np.ones((128, NQH, NKB, 2), dtype=np.float32)
        for qh in range(NQH):
            q0 = qh * QW
            for kb in range(NKB):
                if 4 * qh <= kb < 4 * qh + 4:
                    continue
                k_idx = kidx[kb * 128:(kb + 1) * 128]
                for hh, s in ((0, s0), (1, s1)):
                    if kb < 4 * qh:      # below diag: k < q0
                        rowfac[:, qh, kb, hh] = np.exp(s * (k_idx - q0))
                    else:                # above diag: k >= q0+512
                        rowfac[:, qh, kb, hh] = np.exp(-s * (k_idx - q0 - 511))
        m["rowfac"] = np.ascontiguousarray(rowfac.reshape(128, -1))

    res = run_bass_kernel_spmd(nc, in_maps, list(range(8)), trace=trace)
    _cache["last_res"] = res

    ql = np.arange(QW, dtype=np.float32)
    out = np.empty((B, L, D), dtype=np.float32)
    for c in range(8):
        b = c // HPC
        heads = _core_heads(c)
        oun = res.results[c]["o_un"]  # [HPC, 3, 65, L]
        maskf = mask_np[b].astype(np.float32)
        for hl, Hg in enumerate(heads):
            s = slopes_full[Hg]
            acc = np.empty((65, L), dtype=np.float32)
            for qh in range(NQH):
                sl = slice(qh * QW, (qh + 1) * QW)
                if hl < 2:
                    o_q = oun[hl, 1, :, sl].copy()
                    if qh > 0:
                        o_q += oun[hl, 0, :, sl] * np.exp(-s * ql)[None, :]
                    if qh < NQH - 1:
                        o_q += oun[hl, 2, :, sl] * \
                            np.exp(s * (ql - (QW - 1)))[None, :]
                else:
                    o_q = oun[hl, 1, :, sl]
                acc[:, sl] = o_q
            denom = acc[64, :]
            o_h = (acc[:64, :] / denom[None, :]) * maskf[None, :]
            out[b, :, Hg * HS:(Hg + 1) * HS] = o_h.T
    out += out_bias.reshape(1, 1, D)
    return out


# revision 8
# speedup vs baseline: 1.0427x; 1.0230x over previous
"""Trainium2 Bass kernel for MultiHeadSelfAttention with ALiBi + adjacency bias.

Sharding: 8 cores = 2 batches x 4 pair-groups. Core c (b=c//4, a=c%4) owns
heads [2a, 2a+1, 8+2a, 9+2a]: pair0 = ALiBi heads (slopes 2^-(h+1)),
pair1 = flat heads (slope 0).

Design (all matmuls bf16):
  A) qkvT[c, l] = (W_qk^T @ X^T) (transposed, head-major cols, 1/8 folded
     into Q); V_sb[l, h, hs] = X @ W_v, masked by mask_k; V_aug lhsT
     [k, 66]: col 64 = ones (softmax denom), col 65 = mask_k (masked denom
     so the V input-bias can be applied on host: O += b * maskdenom).
  B) Shared Ea = exp(gamma*adjT) bf16 [2048, 2048] SBUF-RESIDENT (8.4MB,
     loaded once) - replaces the per-head E DMA (was 33.5MB/core).
     ALiBi factor exp(-s|k-q|) decomposes per (qh, kb) tile:
       below-diag (k < q0):        exp(s(k-q0))     * exp(-s(q-q0))
       above-diag (k >= q0+512):   exp(-s(k-q0-511))* exp(s(q-q0-511))
     row part (per-partition k) -> folded into V via tensor_scalar [128,66]
     on the DVE; col part (per-q) -> applied on HOST:
     O accumulated in 3 PSUM phases (below/cross/above), drained
     separately; host combines. Diagonal-crossing tiles use
     host-precomputed Ecross = Ea*exp(-s|k-q|) (bf16, streamed).
  C) per head-pair, per (qh, kb): S^T[k,q] = K Q^T/8 in PSUM fp32
     (concurrent PE row tiles 0-63/64-127), pT = exp(S^T) on ACT
     (PSUM->SBUF bf16, one op for both heads), pb = pT * Ea (DVE bf16,
     broadcast AP reads the 512-wide Ea tile twice), O^T_aug[66,q] +=
     V_aug^T @ pb per phase. Software-pipelined one iteration deep so the
     PE's in-order queue always has the next S-pair ahead of the
     DVE-blocked O-pair, keeping the ACT exp stream back-to-back
     (~1.0us/iter steady state).
  Startup: x DMA split (first 512 seq positions + all weights first), so
  attention starts after only K[kb0-1] + Q[qh0] + V[lb0]; the rest of the
  QKV projection drips in as fillers placed by virtual timestamps.
  Host: combine phase partials with col factors, add V-bias*maskdenom,
  divide by denom, apply mask_q, transpose per-head, assemble, +out_bias.
"""

import math

import numpy as np

B, L, D = 2, 2048, 1024
NH, HS = 16, 64
HPC = 4          # heads per core
NKB = L // 128   # 16 k blocks
QW = 512         # q tile width (1 PSUM bank)
NQH = L // QW    # 4 q tiles
NKC = D // 128   # 8 contraction chunks

_cache = {}


def _alibi_slopes_full():
    ah = NH // 2
    start = 2.0 ** (-(2.0 ** -(math.log2(ah) - 3)))
    s = [start * (start ** i) for i in range(ah)]
    return np.array(s + [0.0] * (NH - ah), dtype=np.float32)


def _core_heads(c):
    a = c % HPC
    return [2 * a, 2 * a + 1, 8 + 2 * a, 9 + 2 * a]


def _build():
    import concourse.tile as tile
    import concourse.mybir as mybir
    from concourse import bacc
    from contextlib import ExitStack

    dt = mybir.dt
    F32, BF16 = dt.float32, dt.bfloat16
    Alu = mybir.AluOpType
    Act = mybir.ActivationFunctionType

    nc = bacc.Bacc("TRN2", target_bir_lowering=False, num_devices=8)

    # xT | wqk | wv concatenated: one DMA per 128-row chunk of D
    xw_d = nc.dram_tensor("xw", [D, L + 512 + 256], BF16, kind="ExternalInput")
    biasqk_d = nc.dram_tensor("biasqk", [128, 4], F32, kind="ExternalInput")
    mask16_d = nc.dram_tensor("mask16", [128, NKB], F32, kind="ExternalInput")
    ea_d = nc.dram_tensor("ea", [128, NKB * L], BF16, kind="ExternalInput")
    ecross_d = nc.dram_tensor(
        "ecross", [NQH, 4, 128, 2 * QW], BF16, kind="ExternalInput")
    rowfac_d = nc.dram_tensor(
        "rowfac", [128, NQH * NKB * 2], F32, kind="ExternalInput")
    oun_d = nc.dram_tensor("o_un", [HPC, 3, 66, L], F32, kind="ExternalOutput")

    with tile.TileContext(nc) as tc, ExitStack() as ctx:
        persist = ctx.enter_context(tc.tile_pool(name="persist", bufs=1))
        # Q^T,K^T bf16: mb 0-1 = Q pairs (h on part 0-63/64-127), 2-3 = K
        qkvT = persist.tile([128, 4, L], BF16)
        # V_aug: [k_part, kb, h, 66] - cols 0:64 = V*mask, 64 = ones, 65 = mask
        vsb = persist.tile([128, NKB, HPC, 66], BF16)
        # shared exp(gamma*adjT): [k_part, kb, q]
        ea = persist.tile([128, NKB, L], BF16)
        rowfac_sb = persist.tile([128, NQH * NKB * 2], F32)

        pa = ctx.enter_context(tc.tile_pool(name="phaseA", bufs=1))
        pe = ctx.enter_context(tc.tile_pool(name="pe", bufs=4))
        pp = ctx.enter_context(tc.tile_pool(name="pp", bufs=8))
        pq = ctx.enter_context(tc.tile_pool(name="pq", bufs=8))
        vp = ctx.enter_context(tc.tile_pool(name="vp", bufs=4))
        outp = ctx.enter_context(tc.tile_pool(name="outp", bufs=4))
        psS = ctx.enter_context(tc.tile_pool(name="psS", bufs=2, space="PSUM"))
        psO = ctx.enter_context(tc.tile_pool(name="psO", bufs=1, space="PSUM"))
        psA = ctx.enter_context(tc.tile_pool(name="psA", bufs=2, space="PSUM"))

        xw_r = pa.tile([128, NKC, L + 512 + 256], BF16)
        xw_dv = xw_d.rearrange("(o p) c -> p o c", p=128)
        # priority DMA: first 512 seq cols of xT + all weight cols
        for kc in range(NKC):
            nc.sync.dma_start(xw_r[:, kc, 0:512], xw_dv[:, kc, 0:512])
            nc.sync.dma_start(
                xw_r[:, kc, L:L + 768], xw_dv[:, kc, L:L + 768])
        biasqk_sb = pa.tile([128, 4], F32)
        nc.sync.dma_start(biasqk_sb[:], biasqk_d[:])
        mask_sb = pa.tile([128, NKB], F32)
        nc.sync.dma_start(mask_sb[:], mask16_d[:])
        nc.sync.dma_start(rowfac_sb[:], rowfac_d[:])
        # crossing-tile E for qh0 (needed by iters 0-3)
        ec_q0 = pa.tile([128, 4, 2 * QW], BF16)
        for j in range(4):
            nc.sync.dma_start(ec_q0[:, j, :], ecross_d[0, j])
        # remaining xT cols
        for kc in range(NKC):
            nc.sync.dma_start(
                xw_r[:, kc, 512:L], xw_dv[:, kc, 512:L])
        # resident Ea: kb 4-7 (first non-cross consumers), 0-3 (below
        # phases of qh1+), then 8-15
        for kb in [4, 5, 6, 7, 0, 1, 2, 3] + list(range(8, NKB)):
            nc.sync.dma_start(ea[:, kb, :], ea_d[:, kb * L:(kb + 1) * L])
        nc.vector.memset(vsb[:, :, :, 64:65], 1.0)
        # col 65 = mask_k (for host-side V-bias: needs masked denominator)
        nc.vector.tensor_copy(
            vsb[:, :, :, 65:66],
            mask_sb[:, :, None, None].broadcast_to([128, NKB, HPC, 1]))
        # tiny dummy exp: pulls the ~2.7us ACT_TABLE_LOAD into the DMA ramp
        wtmp = pa.tile([1, 16], F32)
        nc.vector.memset(wtmp[:], 0.0)
        wex = pa.tile([1, 16], BF16)
        nc.scalar.activation(wex[:], wtmp[:], Act.Exp)

        def t_chunk(mb, c0, c1):
            # qkvT[:, mb, c0:c1] = W_mb^T @ xT[:, c0:c1] (+bias), single
            # accumulation chain (1 PSUM bank)
            def emit():
                ps = psA.tile([128, 512], F32, tag="psA", name="pst")
                for kc in range(NKC):
                    w = xw_r[:, kc, L + mb * 128:L + (mb + 1) * 128]
                    nc.tensor.matmul(
                        ps[:, 0:c1 - c0], w, xw_r[:, kc, c0:c1],
                        start=(kc == 0), stop=(kc == NKC - 1),
                    )
                nc.vector.tensor_scalar(
                    qkvT[:, mb, c0:c1], ps[:, 0:c1 - c0],
                    biasqk_sb[:, mb:mb + 1], None, Alu.add,
                )
            return emit

        def v_chunk(lb):
            # V_sb[l, h*64+hs] = (X @ W_v) * mask_l for all 4 heads; two
            # full-bank PSUM tiles (matmul outputs must be bank-aligned).
            # V input-bias is applied on the host via the masked denom row.
            def emit():
                psva = psA.tile([128, 512], F32, tag="psA", name="psva")
                psvb = psA.tile([128, 512], F32, tag="psA", name="psvb")
                for dc in range(NKC):
                    xc = xw_r[:, dc, lb * 128:(lb + 1) * 128]
                    nc.tensor.matmul(
                        psva[:, 0:128], xc, xw_r[:, dc, L + 512:L + 640],
                        start=(dc == 0), stop=(dc == NKC - 1),
                    )
                    nc.tensor.matmul(
                        psvb[:, 0:128], xc, xw_r[:, dc, L + 640:L + 768],
                        start=(dc == 0), stop=(dc == NKC - 1),
                    )
                for pr, psv in ((0, psva), (1, psvb)):
                    nc.vector.tensor_scalar(
                        vsb[:, lb, 2 * pr:2 * pr + 2, 0:64],
                        psv[:, 0:128].rearrange("p (h c) -> p h c", h=2),
                        mask_sb[:, lb:lb + 1], None, Alu.mult,
                    )
            return emit

        def q_ap(h, c0, c1):
            p0 = (h % 2) * 64
            return qkvT[p0:p0 + 64, h // 2, c0:c1]

        def k_ap(h, c0, c1):
            p0 = (h % 2) * 64
            return qkvT[p0:p0 + 64, 2 + h // 2, c0:c1]

        def attention(pr, fillers=None, psO2=None):
            # One head-pair; see module docstring. Software-pipelined: each
            # iteration's S-pair is emitted before the previous iteration's
            # exp/mult/O tail (PE strict-FIFO: S(k+1) must sit ahead of the
            # DVE-blocked O(k)). psO2: alternate accumulator pool per qh so
            # phase drains overlap with the next phase's matmuls.
            he, ho = 2 * pr, 2 * pr + 1

            def emit_tail(st):
                (qh, slot, kb, ps_s, ope, opo, first, last) = st
                q0 = qh * QW
                pT = pp.tile([128, 2 * QW], BF16, tag="pT")
                nc.scalar.activation(pT[:], ps_s[:], Act.Exp)
                cross = (pr == 0 and slot == 1)
                if pr == 0 and not cross:
                    col = (qh * NKB + kb) * 2
                    vt = vp.tile([128, 2, 66], BF16, tag="vt")
                    nc.vector.tensor_scalar(
                        vt[:, 0, :], vsb[:, kb, he, 0:66],
                        rowfac_sb[:, col:col + 1], None, Alu.mult)
                    nc.vector.tensor_scalar(
                        vt[:, 1, :], vsb[:, kb, ho, 0:66],
                        rowfac_sb[:, col + 1:col + 2], None, Alu.mult)
                    lhs_e, lhs_o = vt[:, 0, :], vt[:, 1, :]
                else:
                    lhs_e = vsb[:, kb, he, 0:66]
                    lhs_o = vsb[:, kb, ho, 0:66]
                pb = pq.tile([128, 2 * QW], BF16, tag="pb")
                if cross:
                    if qh == 0:
                        ec = ec_q0[:, kb, :]
                    else:
                        ect = pe.tile([128, 2 * QW], BF16, tag="ec")
                        nc.sync.dma_start(ect[:], ecross_d[qh, kb - 4 * qh])
                        ec = ect[:]
                    nc.vector.tensor_tensor(pb[:], pT[:], ec, Alu.mult)
                else:
                    ea_b = ea[:, kb, None, q0:q0 + QW].broadcast_to(
                        [128, 2, QW])
                    nc.vector.tensor_tensor(
                        pb[:].rearrange("p (j q) -> p j q", j=2),
                        pT[:].rearrange("p (j q) -> p j q", j=2),
                        ea_b, Alu.mult)
                nc.tensor.matmul(
                    ope[:], lhs_e, pb[:, 0:QW], start=first, stop=last)
                nc.tensor.matmul(
                    opo[:], lhs_o, pb[:, QW:2 * QW], start=first, stop=last)
                if last:
                    for hh, op_t in ((he, ope), (ho, opo)):
                        ot = outp.tile([66, QW], F32, tag="ot")
                        nc.vector.tensor_copy(ot[:], op_t[:])
                        nc.sync.dma_start(
                            oun_d[hh, slot, :, q0:q0 + QW], ot[:])
                if fillers:
                    g = qh * NKB + kb
                    if g < 16:
                        est_ms = (8.0 + 2.2 * g) * 1e-3
                    else:
                        est_ms = (43.0 + 1.35 * (g - 16)) * 1e-3
                    for fn in fillers.get((qh, kb), ()):
                        with tc.tile_wait_until(est_ms):
                            fn()

            pending = None
            for qh in range(NQH):
                q0 = qh * QW
                if pr == 0:
                    phases = []
                    if qh > 0:
                        phases.append((0, list(range(0, 4 * qh))))
                    phases.append((1, list(range(4 * qh, 4 * qh + 4))))
                    if qh < NQH - 1:
                        phases.append((2, list(range(4 * qh + 4, NKB))))
                else:
                    phases = [(1, list(range(NKB)))]
                for slot, kbs in phases:
                    pool = psO if (psO2 is None or qh % 2 == 0) else psO2
                    ope = pool.tile([66, QW], F32, tag="ope", name="ope")
                    opo = pool.tile([66, QW], F32, tag="opo", name="opo")
                    for i, kb in enumerate(kbs):
                        first, last = (i == 0), (i == len(kbs) - 1)
                        ps_s = psS.tile([128, 2 * QW], F32, tag="ps_s")
                        nc.tensor.matmul(
                            ps_s[:, 0:QW],
                            k_ap(he, kb * 128, (kb + 1) * 128),
                            q_ap(he, q0, q0 + QW), start=True, stop=True,
                        )
                        nc.tensor.matmul(
                            ps_s[:, QW:2 * QW],
                            k_ap(ho, kb * 128, (kb + 1) * 128),
                            q_ap(ho, q0, q0 + QW), start=True, stop=True,
                        )
                        if pending is not None:
                            emit_tail(pending)
                        pending = (qh, slot, kb, ps_s, ope, opo, first, last)
            emit_tail(pending)

        # Narrow head: only what iteration (qh0, kb0) needs, then start
        # attention; everything else drips in as fillers.
        t_chunk(2, 0, 256)()         # K pair0 kb0-1
        t_chunk(0, 0, 512)()         # Q pair0 qh0
        v_chunk(0)()
        fillers = {}

        def put(qh, kb, chunk):
            fillers.setdefault((qh, kb), []).append(chunk)

        # K pair0: block kb needed at iter (0, kb); 2-block chunks
        for i, kb0 in enumerate(range(2, NKB, 2)):   # (2,3),(4,5),...,(14,15)
            put(0, max(0, kb0 - 2), t_chunk(2, kb0 * 128, (kb0 + 2) * 128))
        # V: block lb needed at iter (0, lb)
        for lb in range(1, NKB):
            put(0, max(0, lb - 2), v_chunk(lb))
        # Q pair0 qh1-3: needed at iters 16/32/48
        put(0, 10, t_chunk(0, 512, 1024))
        put(1, 8, t_chunk(0, 1024, 1536))
        put(2, 8, t_chunk(0, 1536, 2048))
        # K pair1: needed from iter 64
        put(2, 12, t_chunk(3, 0, 512))
        put(2, 14, t_chunk(3, 512, 1024))
        put(3, 0, t_chunk(3, 1024, 1536))
        put(3, 2, t_chunk(3, 1536, 2048))
        # Q pair1: needed at iters 64/80/96/112
        put(3, 4, t_chunk(1, 0, 512))
        put(3, 6, t_chunk(1, 512, 1024))
        put(3, 8, t_chunk(1, 1024, 1536))
        put(3, 10, t_chunk(1, 1536, 2048))
        attention(0, fillers)
        attention(1)

    nc.compile()
    return nc


def _prep_inputs(x, adj, mask, weights, in_bias):
    import ml_dtypes
    bf16 = ml_dtypes.bfloat16

    wq = np.array(weights, dtype=np.float32, copy=True)
    bq = np.array(in_bias, dtype=np.float32, copy=True).reshape(3 * D)
    for h in range(NH):
        wq[:, h * 192:h * 192 + 64] *= 0.125
        bq[h * 192:h * 192 + 64] *= 0.125

    in_maps = []
    for c in range(8):
        b = c // HPC
        heads = _core_heads(c)
        # QK cols: [Q_h0..Q_h3 | K_h0..K_h3], V cols: [V_h0..V_h3]
        perm_qk = np.concatenate([
            np.arange(H * 192 + which * 64, H * 192 + which * 64 + 64)
            for which in range(2) for H in heads
        ])
        perm_v = np.concatenate([
            np.arange(H * 192 + 128, H * 192 + 192) for H in heads
        ])
        xw = np.ascontiguousarray(np.concatenate(
            [x[b].T, wq[:, perm_qk], wq[:, perm_v]], axis=1)).astype(bf16)
        biasqk = np.ascontiguousarray(bq[perm_qk].reshape(4, 128).T)
        maskf = mask[b].astype(np.float32)
        mask16 = np.ascontiguousarray(maskf.reshape(NKB, 128).T)
        in_maps.append({
            "xw": xw, "biasqk": biasqk, "mask16": mask16,
            "ea": None, "ecross": None, "rowfac": None,  # filled in kernel()
            "_b": b, "_heads": heads,
        })
    return in_maps


def _reference_numpy(x, adj, mask, weights, in_bias, out_bias, gamma):
    # correct fallback for inputs the fast path doesn't cover
    slopes = _alibi_slopes_full()
    pos = np.arange(L, dtype=np.float32)
    rel = -np.abs(pos[None, :] - pos[:, None])
    out = np.empty((B, L, D), dtype=np.float32)
    qkv = x @ weights + in_bias.reshape(1, 1, 3 * D)
    gamma = gamma.reshape(NH)
    for b in range(B):
        for h in range(NH):
            q = qkv[b, :, h * 192:h * 192 + 64]
            k = qkv[b, :, h * 192 + 64:h * 192 + 128]
            v = qkv[b, :, h * 192 + 128:h * 192 + 192]
            s = q @ k.T / 8.0 + slopes[h] * rel + gamma[h] * adj[b, 0]
            s = s - s.max(axis=1, keepdims=True)
            p = np.exp(s)
            p /= p.sum(axis=1, keepdims=True)
            m2 = (mask[b][:, None] & mask[b][None, :]).astype(np.float32)
            out[b, :, h * 64:(h + 1) * 64] = (p * m2) @ v
    return out + out_bias.reshape(1, 1, D)


def kernel(x, adj, mask, weights, in_bias, out_bias, gamma):
    import os
    import ml_dtypes
    from concourse.bass_utils import run_bass_kernel_spmd

    bf16 = ml_dtypes.bfloat16

    x = np.asarray(x, dtype=np.float32)
    adj = np.asarray(adj, dtype=np.float32)
    mask_np = np.asarray(mask)
    weights = np.asarray(weights, dtype=np.float32)
    in_bias = np.asarray(in_bias, dtype=np.float32)
    out_bias = np.asarray(out_bias, dtype=np.float32)
    gamma_np = np.asarray(gamma, dtype=np.float32).reshape(NH)
    slopes_full = _alibi_slopes_full()

    if not np.all(gamma_np == gamma_np[0]):
        # shared-Ea fast path needs uniform gamma; fall back to exact host
        return _reference_numpy(
            x, adj, mask_np, weights, in_bias, out_bias,
            np.asarray(gamma, dtype=np.float32))
    g0 = float(gamma_np[0])

    if "nc" not in _cache:
        _cache["nc"] = _build()
    nc = _cache["nc"]
    trace = os.environ.get("BASS_TRACE", "0") == "1"

    in_maps = _prep_inputs(x, adj, mask_np, weights, in_bias)
    bv = in_bias.reshape(3 * D)  # V bias slice per head: [h*192+128, +64)

    kidx = np.arange(L, dtype=np.float32)
    ea_by_b = [np.exp(g0 * adj[b, 0].T).astype(np.float32) for b in range(B)]

    for c, m in enumerate(in_maps):
        b, heads = m.pop("_b"), m.pop("_heads")
        ea_f = ea_by_b[b]
        # device layout [p, kb, q] -> flat [128, NKB*L]
        m["ea"] = np.ascontiguousarray(
            ea_f.reshape(NKB, 128, L).transpose(1, 0, 2).reshape(128, NKB * L)
        ).astype(bf16)

        # Ecross[qh, j, p, hh*QW+ql] for the ALiBi pair (local heads 0,1)
        s0, s1 = slopes_full[heads[0]], slopes_full[heads[1]]
        ecross = np.empty((NQH, 4, 128, 2 * QW), dtype=bf16)
        for qh in range(NQH):
            q_idx = kidx[qh * QW:(qh + 1) * QW]
            for j in range(4):
                kb = 4 * qh + j
                k_idx = kidx[kb * 128:(kb + 1) * 128]
                absd = np.abs(k_idx[:, None] - q_idx[None, :])
                base = ea_f[kb * 128:(kb + 1) * 128, qh * QW:(qh + 1) * QW]
                ecross[qh, j, :, 0:QW] = (base * np.exp(-s0 * absd))
                ecross[qh, j, :, QW:] = (base * np.exp(-s1 * absd))
        m["ecross"] = ecross

        # rowfac[p, ((qh*NKB+kb)*2 + hh)] fp32
        rowfac = np.ones((128, NQH, NKB, 2), dtype=np.float32)
        for qh in range(NQH):
            q0 = qh * QW
            for kb in range(NKB):
                if 4 * qh <= kb < 4 * qh + 4:
                    continue
                k_idx = kidx[kb * 128:(kb + 1) * 128]
                for hh, s in ((0, s0), (1, s1)):
                    if kb < 4 * qh:      # below diag: k < q0
                        rowfac[:, qh, kb, hh] = np.exp(s * (k_idx - q0))
                    else:                # above diag: k >= q0+512
                        rowfac[:, qh, kb, hh] = np.exp(-s * (k_idx - q0 - 511))
        m["rowfac"] = np.ascontiguousarray(rowfac.reshape(128, -1))

    res = run_bass_kernel_spmd(nc, in_maps, list(range(8)), trace=trace)
    _cache["last_res"] = res

    ql = np.arange(QW, dtype=np.float32)
    out = np.empty((B, L, D), dtype=np.float32)
    for c in range(8):
        b = c // HPC
        heads = _core_heads(c)
        oun = res.results[c]["o_un"]  # [HPC, 3, 66, L]
        maskf = mask_np[b].astype(np.float32)
        for hl, Hg in enumerate(heads):
            s = slopes_full[Hg]
            acc = np.empty((66, L), dtype=np.float32)
            for qh in range(NQH):
                sl = slice(qh * QW, (qh + 1) * QW)
                if hl < 2:
                    o_q = oun[hl, 1, :, sl].copy()
                    if qh > 0:
                        o_q += oun[hl, 0, :, sl] * np.exp(-s * ql)[None, :]
                    if qh < NQH - 1:
                        o_q += oun[hl, 2, :, sl] * \
                            np.exp(s * (ql - (QW - 1)))[None, :]
                else:
                    o_q = oun[hl, 1, :, sl]
                acc[:, sl] = o_q
            denom = acc[64, :]
            bvh = bv[Hg * 192 + 128:Hg * 192 + 192]  # V input-bias
            num = acc[:64, :] + bvh[:, None] * acc[65:66, :]
            o_h = (num / denom[None, :]) * maskf[None, :]
            out[b, :, Hg * HS:(Hg + 1) * HS] = o_h.T
    out += out_bias.reshape(1, 1, D)
    return out


# revision 15
# speedup vs baseline: 1.0577x; 1.0143x over previous
"""Trainium2 Bass kernel for MultiHeadSelfAttention with ALiBi + adjacency bias.

Sharding: 8 cores = 2 batches x 4 pair-groups. Core c (b=c//4, a=c%4) owns
heads [2a, 2a+1, 8+2a, 9+2a]: pair0 = ALiBi heads (slopes 2^-(h+1)),
pair1 = flat heads (slope 0).

Design (all matmuls bf16):
  A) qkvT[c, l] = (W_qk^T @ X^T) (transposed, head-major cols, 1/8 folded
     into Q); V_sb[l, h, hs] = X @ W_v, masked by mask_k; V_aug lhsT
     [k, 66]: col 64 = ones (softmax denom), col 65 = mask_k (masked denom
     so the V input-bias can be applied on host: O += b * maskdenom).
  B) Shared Ea = exp(gamma*adjT) bf16 [2048, 2048] SBUF-RESIDENT (8.4MB,
     loaded once) - replaces the per-head E DMA (was 33.5MB/core).
     ALiBi factor exp(-s|k-q|) decomposes per (qh, kb) tile:
       below-diag (k < q0):        exp(s(k-q0))     * exp(-s(q-q0))
       above-diag (k >= q0+512):   exp(-s(k-q0-511))* exp(s(q-q0-511))
     row part (per-partition k) -> folded into V via tensor_scalar [128,66]
     on the DVE; col part (per-q) -> applied on HOST:
     O accumulated in 3 PSUM phases (below/cross/above), drained
     separately; host combines. Diagonal-crossing tiles use
     host-precomputed Ecross = Ea*exp(-s|k-q|) (bf16, streamed).
  C) per head-pair, per (qh, kb): S^T[k,q] = K Q^T/8 in PSUM fp32
     (concurrent PE row tiles 0-63/64-127), pT = exp(S^T) on ACT
     (PSUM->SBUF bf16, one op for both heads), pb = pT * Ea (DVE bf16,
     broadcast AP reads the 512-wide Ea tile twice), O^T_aug[66,q] +=
     V_aug^T @ pb per phase. Software-pipelined one iteration deep so the
     PE's in-order queue always has the next S-pair ahead of the
     DVE-blocked O-pair, keeping the ACT exp stream back-to-back
     (~1.0us/iter steady state).
  Startup: x DMA split (first 512 seq positions + all weights first), so
  attention starts after only K[kb0-1] + Q[qh0] + V[lb0]; the rest of the
  QKV projection drips in as fillers placed by virtual timestamps.
  Host: combine phase partials with col factors, add V-bias*maskdenom,
  divide by denom, apply mask_q, transpose per-head, assemble, +out_bias.
"""

import math

import numpy as np

B, L, D = 2, 2048, 1024
NH, HS = 16, 64
HPC = 4          # heads per core
NKB = L // 128   # 16 k blocks
QW = 512         # q tile width (1 PSUM bank)
NQH = L // QW    # 4 q tiles
NKC = D // 128   # 8 contraction chunks

_cache = {}


def _alibi_slopes_full():
    ah = NH // 2
    start = 2.0 ** (-(2.0 ** -(math.log2(ah) - 3)))
    s = [start * (start ** i) for i in range(ah)]
    return np.array(s + [0.0] * (NH - ah), dtype=np.float32)


def _core_heads(c):
    a = c % HPC
    return [2 * a, 2 * a + 1, 8 + 2 * a, 9 + 2 * a]


def _build():
    import concourse.tile as tile
    import concourse.mybir as mybir
    from concourse import bacc
    from contextlib import ExitStack

    dt = mybir.dt
    F32, BF16 = dt.float32, dt.bfloat16
    Alu = mybir.AluOpType
    Act = mybir.ActivationFunctionType

    nc = bacc.Bacc("TRN2", target_bir_lowering=False, num_devices=8)

    # xT | wqk | wv concatenated: one DMA per 128-row chunk of D
    xw_d = nc.dram_tensor("xw", [D, L + 512 + 256], BF16, kind="ExternalInput")
    biasqk_d = nc.dram_tensor("biasqk", [128, 4], F32, kind="ExternalInput")
    mask16_d = nc.dram_tensor("mask16", [128, NKB], F32, kind="ExternalInput")
    ea_d = nc.dram_tensor("ea", [128, NKB * L], BF16, kind="ExternalInput")
    ecross_d = nc.dram_tensor(
        "ecross", [NQH, 4, 128, 2 * QW], BF16, kind="ExternalInput")
    rowfac_d = nc.dram_tensor(
        "rowfac", [128, NQH * NKB * 2], F32, kind="ExternalInput")
    oun_d = nc.dram_tensor("o_un", [HPC, 3, 66, L], F32, kind="ExternalOutput")

    with tile.TileContext(nc) as tc, ExitStack() as ctx:
        persist = ctx.enter_context(tc.tile_pool(name="persist", bufs=1))
        # Q^T,K^T bf16: mb 0-1 = Q pairs (h on part 0-63/64-127), 2-3 = K
        qkvT = persist.tile([128, 4, L], BF16)
        # V_aug: [k_part, kb, h, 66] - cols 0:64 = V*mask, 64 = ones, 65 = mask
        vsb = persist.tile([128, NKB, HPC, 66], BF16)
        # shared exp(gamma*adjT): [k_part, kb, q]
        ea = persist.tile([128, NKB, L], BF16)
        rowfac_sb = persist.tile([128, NQH * NKB * 2], F32)

        pa = ctx.enter_context(tc.tile_pool(name="phaseA", bufs=1))
        pe = ctx.enter_context(tc.tile_pool(name="pe", bufs=4))
        pp = ctx.enter_context(tc.tile_pool(name="pp", bufs=8))
        pq = ctx.enter_context(tc.tile_pool(name="pq", bufs=8))
        vp = ctx.enter_context(tc.tile_pool(name="vp", bufs=4))
        outp = ctx.enter_context(tc.tile_pool(name="outp", bufs=4))
        psS = ctx.enter_context(tc.tile_pool(name="psS", bufs=2, space="PSUM"))
        psO = ctx.enter_context(tc.tile_pool(name="psO", bufs=1, space="PSUM"))
        psA = ctx.enter_context(tc.tile_pool(name="psA", bufs=2, space="PSUM"))

        # tiny dummy exp FIRST: pulls the ~2.7us ACT_TABLE_LOAD into the
        # DMA ramp (must not sit behind DMA-waiting DVE ops)
        wtmp = pa.tile([1, 16], F32)
        nc.vector.memset(wtmp[:], 0.0)
        wex = pa.tile([1, 16], BF16)
        nc.scalar.activation(wex[:], wtmp[:], Act.Exp)
        # small inputs first so their consumers don't queue behind bulk DMA
        biasqk_sb = pa.tile([128, 4], F32)
        nc.sync.dma_start(biasqk_sb[:], biasqk_d[:])
        mask_sb = pa.tile([128, NKB], F32)
        nc.sync.dma_start(mask_sb[:], mask16_d[:])
        nc.sync.dma_start(rowfac_sb[:], rowfac_d[:])
        # xT/W as THREE tiles so dependency tracking (tile-granular) lets
        # the first QKV chains start after only x[0:512]+W has landed
        xw_hi = pa.tile([128, NKC, 512], BF16)   # xT cols 0:512
        xw_w = pa.tile([128, NKC, 768], BF16)    # Wqk|Wv cols
        xw_lo = pa.tile([128, NKC, 1536], BF16)  # xT cols 512:2048
        xw_dv = xw_d.rearrange("(o p) c -> p o c", p=128)
        for kc in range(NKC):
            nc.sync.dma_start(xw_hi[:, kc, :], xw_dv[:, kc, 0:512])
            nc.sync.dma_start(xw_w[:, kc, :], xw_dv[:, kc, L:L + 768])
        # crossing-tile E for qh0 (needed by iters 0-3)
        ec_q0 = pa.tile([128, 4, 2 * QW], BF16)
        for j in range(4):
            nc.sync.dma_start(ec_q0[:, j, :], ecross_d[0, j])
        # remaining xT cols
        for kc in range(NKC):
            nc.sync.dma_start(xw_lo[:, kc, :], xw_dv[:, kc, 512:L])
        # resident Ea: kb 4-7 (first non-cross consumers), 0-3 (below
        # phases of qh1+), then 8-15
        for kb in [4, 5, 6, 7, 0, 1, 2, 3] + list(range(8, NKB)):
            nc.sync.dma_start(ea[:, kb, :], ea_d[:, kb * L:(kb + 1) * L])
        nc.vector.memset(vsb[:, :, :, 64:65], 1.0)
        # col 65 = mask_k (for host-side V-bias: needs masked denominator)
        nc.vector.tensor_copy(
            vsb[:, :, :, 65:66],
            mask_sb[:, :, None, None].broadcast_to([128, NKB, HPC, 1]))

        def x_ap(kc, c0, c1):
            # xT column range [c0, c1) from the split tiles (no straddling)
            if c1 <= 512:
                return xw_hi[:, kc, c0:c1]
            assert c0 >= 512
            return xw_lo[:, kc, c0 - 512:c1 - 512]

        def t_chunk(mb, c0, c1):
            # qkvT[:, mb, c0:c1] = W_mb^T @ xT[:, c0:c1] (+bias), single
            # accumulation chain (1 PSUM bank)
            def emit():
                ps = psA.tile([128, 512], F32, tag="psA", name="pst")
                for kc in range(NKC):
                    w = xw_w[:, kc, mb * 128:(mb + 1) * 128]
                    nc.tensor.matmul(
                        ps[:, 0:c1 - c0], w, x_ap(kc, c0, c1),
                        start=(kc == 0), stop=(kc == NKC - 1),
                    )
                nc.vector.tensor_scalar(
                    qkvT[:, mb, c0:c1], ps[:, 0:c1 - c0],
                    biasqk_sb[:, mb:mb + 1], None, Alu.add,
                )
            return emit

        def v_chunk(lb):
            # V_sb[l, h*64+hs] = (X @ W_v) * mask_l for all 4 heads; two
            # full-bank PSUM tiles (matmul outputs must be bank-aligned).
            # V input-bias is applied on the host via the masked denom row.
            def emit():
                psva = psA.tile([128, 512], F32, tag="psA", name="psva")
                psvb = psA.tile([128, 512], F32, tag="psA", name="psvb")
                for dc in range(NKC):
                    xc = x_ap(dc, lb * 128, (lb + 1) * 128)
                    nc.tensor.matmul(
                        psva[:, 0:128], xc, xw_w[:, dc, 512:640],
                        start=(dc == 0), stop=(dc == NKC - 1),
                    )
                    nc.tensor.matmul(
                        psvb[:, 0:128], xc, xw_w[:, dc, 640:768],
                        start=(dc == 0), stop=(dc == NKC - 1),
                    )
                for pr, psv in ((0, psva), (1, psvb)):
                    nc.vector.tensor_scalar(
                        vsb[:, lb, 2 * pr:2 * pr + 2, 0:64],
                        psv[:, 0:128].rearrange("p (h c) -> p h c", h=2),
                        mask_sb[:, lb:lb + 1], None, Alu.mult,
                    )
            return emit

        def q_ap(h, c0, c1):
            p0 = (h % 2) * 64
            return qkvT[p0:p0 + 64, h // 2, c0:c1]

        def k_ap(h, c0, c1):
            p0 = (h % 2) * 64
            return qkvT[p0:p0 + 64, 2 + h // 2, c0:c1]

        def attention(pr, fillers=None, psO2=None):
            # One head-pair; see module docstring. Software-pipelined: each
            # iteration's S-pair is emitted before the previous iteration's
            # exp/mult/O tail (PE strict-FIFO: S(k+1) must sit ahead of the
            # DVE-blocked O(k)). psO2: alternate accumulator pool per qh so
            # phase drains overlap with the next phase's matmuls.
            he, ho = 2 * pr, 2 * pr + 1

            def emit_tail(st):
                (qh, slot, kb, ps_s, ope, opo, first, last) = st
                q0 = qh * QW
                pT = pp.tile([128, 2 * QW], BF16, tag="pT")
                nc.scalar.activation(pT[:], ps_s[:], Act.Exp)
                cross = (pr == 0 and slot == 1)
                if pr == 0 and not cross:
                    col = (qh * NKB + kb) * 2
                    vt = vp.tile([128, 2, 66], BF16, tag="vt")
                    nc.vector.tensor_scalar(
                        vt[:, 0, :], vsb[:, kb, he, 0:66],
                        rowfac_sb[:, col:col + 1], None, Alu.mult)
                    nc.vector.tensor_scalar(
                        vt[:, 1, :], vsb[:, kb, ho, 0:66],
                        rowfac_sb[:, col + 1:col + 2], None, Alu.mult)
                    lhs_e, lhs_o = vt[:, 0, :], vt[:, 1, :]
                else:
                    lhs_e = vsb[:, kb, he, 0:66]
                    lhs_o = vsb[:, kb, ho, 0:66]
                pb = pq.tile([128, 2 * QW], BF16, tag="pb")
                if cross:
                    if qh == 0:
                        ec = ec_q0[:, kb, :]
                    else:
                        ect = pe.tile([128, 2 * QW], BF16, tag="ec")
                        nc.sync.dma_start(ect[:], ecross_d[qh, kb - 4 * qh])
                        ec = ect[:]
                    nc.vector.tensor_tensor(pb[:], pT[:], ec, Alu.mult)
                else:
                    ea_b = ea[:, kb, None, q0:q0 + QW].broadcast_to(
                        [128, 2, QW])
                    nc.vector.tensor_tensor(
                        pb[:].rearrange("p (j q) -> p j q", j=2),
                        pT[:].rearrange("p (j q) -> p j q", j=2),
                        ea_b, Alu.mult)
                nc.tensor.matmul(
                    ope[:], lhs_e, pb[:, 0:QW], start=first, stop=last)
                nc.tensor.matmul(
                    opo[:], lhs_o, pb[:, QW:2 * QW], start=first, stop=last)
                if last:
                    for hh, op_t in ((he, ope), (ho, opo)):
                        ot = outp.tile([66, QW], F32, tag="ot")
                        nc.vector.tensor_copy(ot[:], op_t[:])
                        nc.sync.dma_start(
                            oun_d[hh, slot, :, q0:q0 + QW], ot[:])
                if fillers:
                    g = qh * NKB + kb
                    if pr == 0:
                        if g < 16:
                            est_ms = (9.0 + 2.4 * g) * 1e-3
                        else:
                            est_ms = (48.0 + 1.3 * (g - 16)) * 1e-3
                    else:
                        est_ms = (108.0 + 1.1 * g) * 1e-3
                    for fn in fillers.get((qh, kb), ()):
                        with tc.tile_wait_until(est_ms):
                            fn()

            pending = None
            for qh in range(NQH):
                q0 = qh * QW
                if pr == 0:
                    phases = []
                    if qh > 0:
                        phases.append((0, list(range(0, 4 * qh))))
                    phases.append((1, list(range(4 * qh, 4 * qh + 4))))
                    if qh < NQH - 1:
                        phases.append((2, list(range(4 * qh + 4, NKB))))
                else:
                    phases = [(1, list(range(NKB)))]
                for slot, kbs in phases:
                    if pr == 1 and qh % 2 == 1:
                        # pair1 odd qh: borrow the (filler-retired) psA
                        # buffers so drains overlap the next qh's matmuls
                        ope = psA.tile([66, QW], F32, tag="psA", name="ope2")
                        opo = psA.tile([66, QW], F32, tag="psA", name="opo2")
                    else:
                        ope = psO.tile([66, QW], F32, tag="ope", name="ope")
                        opo = psO.tile([66, QW], F32, tag="opo", name="opo")
                    for i, kb in enumerate(kbs):
                        first, last = (i == 0), (i == len(kbs) - 1)
                        ps_s = psS.tile([128, 2 * QW], F32, tag="ps_s")
                        nc.tensor.matmul(
                            ps_s[:, 0:QW],
                            k_ap(he, kb * 128, (kb + 1) * 128),
                            q_ap(he, q0, q0 + QW), start=True, stop=True,
                        )
                        nc.tensor.matmul(
                            ps_s[:, QW:2 * QW],
                            k_ap(ho, kb * 128, (kb + 1) * 128),
                            q_ap(ho, q0, q0 + QW), start=True, stop=True,
                        )
                        if pending is not None:
                            emit_tail(pending)
                        pending = (qh, slot, kb, ps_s, ope, opo, first, last)
            emit_tail(pending)

        # Narrow head: only what iteration (qh0, kb0) needs, then start
        # attention; everything else drips in as fillers.
        t_chunk(2, 0, 256)()         # K pair0 kb0-1
        t_chunk(0, 0, 512)()         # Q pair0 qh0
        v_chunk(0)()
        fillers = {}

        def put(qh, kb, chunk):
            fillers.setdefault((qh, kb), []).append(chunk)

        # K pair0: block kb needed at iter (0, kb); 2-block chunks
        for i, kb0 in enumerate(range(2, NKB, 2)):   # (2,3),(4,5),...,(14,15)
            put(0, max(0, kb0 - 2), t_chunk(2, kb0 * 128, (kb0 + 2) * 128))
        # V: block lb needed at iter (0, lb)
        for lb in range(1, NKB):
            put(0, max(0, lb - 2), v_chunk(lb))
        # Q pair0 qh1-3: needed at iters 16/32/48
        put(0, 10, t_chunk(0, 512, 1024))
        put(1, 8, t_chunk(0, 1024, 1536))
        put(2, 8, t_chunk(0, 1536, 2048))
        # K pair1 kb0-7 + Q pair1 qh0: must finish before pair1 starts
        put(2, 12, t_chunk(3, 0, 512))
        put(3, 0, t_chunk(3, 512, 1024))
        put(3, 6, t_chunk(1, 0, 512))
        attention(0, fillers)
        # pair1's remaining K/Q drip in during pair1's own ACT-bound
        # iterations (PE has ~15% slack there). Even qhs only: odd qhs'
        # accumulators borrow the psA buffers these chunks would need.
        fillers1 = {}
        fillers1[(0, 2)] = [t_chunk(3, 1024, 1536)]   # K kb8-11 (iter 72)
        fillers1[(0, 8)] = [t_chunk(3, 1536, 2048)]   # K kb12-15 (iter 76)
        fillers1[(0, 12)] = [t_chunk(1, 512, 1024)]   # Q qh1 (iter 80)
        fillers1[(0, 14)] = [t_chunk(1, 1024, 1536)]  # Q qh2 (iter 96)
        fillers1[(2, 2)] = [t_chunk(1, 1536, 2048)]   # Q qh3 (iter 112)
        attention(1, fillers1)

    nc.compile()
    return nc


def _prep_inputs(x, adj, mask, weights, in_bias):
    import ml_dtypes
    bf16 = ml_dtypes.bfloat16

    wq = np.array(weights, dtype=np.float32, copy=True)
    bq = np.array(in_bias, dtype=np.float32, copy=True).reshape(3 * D)
    for h in range(NH):
        wq[:, h * 192:h * 192 + 64] *= 0.125
        bq[h * 192:h * 192 + 64] *= 0.125

    in_maps = []
    for c in range(8):
        b = c // HPC
        heads = _core_heads(c)
        # QK cols: [Q_h0..Q_h3 | K_h0..K_h3], V cols: [V_h0..V_h3]
        perm_qk = np.concatenate([
            np.arange(H * 192 + which * 64, H * 192 + which * 64 + 64)
            for which in range(2) for H in heads
        ])
        perm_v = np.concatenate([
            np.arange(H * 192 + 128, H * 192 + 192) for H in heads
        ])
        xw = np.ascontiguousarray(np.concatenate(
            [x[b].T, wq[:, perm_qk], wq[:, perm_v]], axis=1)).astype(bf16)
        biasqk = np.ascontiguousarray(bq[perm_qk].reshape(4, 128).T)
        maskf = mask[b].astype(np.float32)
        mask16 = np.ascontiguousarray(maskf.reshape(NKB, 128).T)
        in_maps.append({
            "xw": xw, "biasqk": biasqk, "mask16": mask16,
            "ea": None, "ecross": None, "rowfac": None,  # filled in kernel()
            "_b": b, "_heads": heads,
        })
    return in_maps


def _reference_numpy(x, adj, mask, weights, in_bias, out_bias, gamma):
    # correct fallback for inputs the fast path doesn't cover
    slopes = _alibi_slopes_full()
    pos = np.arange(L, dtype=np.float32)
    rel = -np.abs(pos[None, :] - pos[:, None])
    out = np.empty((B, L, D), dtype=np.float32)
    qkv = x @ weights + in_bias.reshape(1, 1, 3 * D)
    gamma = gamma.reshape(NH)
    for b in range(B):
        for h in range(NH):
            q = qkv[b, :, h * 192:h * 192 + 64]
            k = qkv[b, :, h * 192 + 64:h * 192 + 128]
            v = qkv[b, :, h * 192 + 128:h * 192 + 192]
            s = q @ k.T / 8.0 + slopes[h] * rel + gamma[h] * adj[b, 0]
            s = s - s.max(axis=1, keepdims=True)
            p = np.exp(s)
            p /= p.sum(axis=1, keepdims=True)
            m2 = (mask[b][:, None] & mask[b][None, :]).astype(np.float32)
            out[b, :, h * 64:(h + 1) * 64] = (p * m2) @ v
    return out + out_bias.reshape(1, 1, D)


def kernel(x, adj, mask, weights, in_bias, out_bias, gamma):
    import os
    import ml_dtypes
    from concourse.bass_utils import run_bass_kernel_spmd

    bf16 = ml_dtypes.bfloat16

    x = np.asarray(x, dtype=np.float32)
    adj = np.asarray(adj, dtype=np.float32)
    mask_np = np.asarray(mask)
    weights = np.asarray(weights, dtype=np.float32)
    in_bias = np.asarray(in_bias, dtype=np.float32)
    out_bias = np.asarray(out_bias, dtype=np.float32)
    gamma_np = np.asarray(gamma, dtype=np.float32).reshape(NH)
    slopes_full = _alibi_slopes_full()

    if not np.all(gamma_np == gamma_np[0]):
        # shared-Ea fast path needs uniform gamma; fall back to exact host
        return _reference_numpy(
            x, adj, mask_np, weights, in_bias, out_bias,
            np.asarray(gamma, dtype=np.float32))
    g0 = float(gamma_np[0])

    if "nc" not in _cache:
        _cache["nc"] = _build()
    nc = _cache["nc"]
    trace = os.environ.get("BASS_TRACE", "0") == "1"

    in_maps = _prep_inputs(x, adj, mask_np, weights, in_bias)
    bv = in_bias.reshape(3 * D)  # V bias slice per head: [h*192+128, +64)

    kidx = np.arange(L, dtype=np.float32)
    ea_by_b = [np.exp(g0 * adj[b, 0].T).astype(np.float32) for b in range(B)]

    for c, m in enumerate(in_maps):
        b, heads = m.pop("_b"), m.pop("_heads")
        ea_f = ea_by_b[b]
        # device layout [p, kb, q] -> flat [128, NKB*L]
        m["ea"] = np.ascontiguousarray(
            ea_f.reshape(NKB, 128, L).transpose(1, 0, 2).reshape(128, NKB * L)
        ).astype(bf16)

        # Ecross[qh, j, p, hh*QW+ql] for the ALiBi pair (local heads 0,1)
        s0, s1 = slopes_full[heads[0]], slopes_full[heads[1]]
        ecross = np.empty((NQH, 4, 128, 2 * QW), dtype=bf16)
        for qh in range(NQH):
            q_idx = kidx[qh * QW:(qh + 1) * QW]
            for j in range(4):
                kb = 4 * qh + j
                k_idx = kidx[kb * 128:(kb + 1) * 128]
                absd = np.abs(k_idx[:, None] - q_idx[None, :])
                base = ea_f[kb * 128:(kb + 1) * 128, qh * QW:(qh + 1) * QW]
                ecross[qh, j, :, 0:QW] = (base * np.exp(-s0 * absd))
                ecross[qh, j, :, QW:] = (base * np.exp(-s1 * absd))
        m["ecross"] = ecross

        # rowfac[p, ((qh*NKB+kb)*2 + hh)] fp32
        rowfac = np.ones((128, NQH, NKB, 2), dtype=np.float32)
        for qh in range(NQH):
            q0 = qh * QW
            for kb in range(NKB):
                if 4 * qh <= kb < 4 * qh + 4:
                    continue
                k_idx = kidx[kb * 128:(kb + 1) * 128]
                for hh, s in ((0, s0), (1, s1)):
                    if kb < 4 * qh:      # below diag: k < q0
                        rowfac[:, qh, kb, hh] = np.exp(s * (k_idx - q0))
                    else:                # above diag: k >= q0+512
                        rowfac[:, qh, kb, hh] = np.exp(-s * (k_idx - q0 - 511))
        m["rowfac"] = np.ascontiguousarray(rowfac.reshape(128, -1))

    res = run_bass_kernel_spmd(nc, in_maps, list(range(8)), trace=trace)
    _cache["last_res"] = res

    ql = np.arange(QW, dtype=np.float32)
    out = np.empty((B, L, D), dtype=np.float32)
    for c in range(8):
        b = c // HPC
        heads = _core_heads(c)
        oun = res.results[c]["o_un"]  # [HPC, 3, 66, L]
        maskf = mask_np[b].astype(np.float32)
        for hl, Hg in enumerate(heads):
            s = slopes_full[Hg]
            acc = np.empty((66, L), dtype=np.float32)
            for qh in range(NQH):
                sl = slice(qh * QW, (qh + 1) * QW)
                if hl < 2:
                    o_q = oun[hl, 1, :, sl].copy()
                    if qh > 0:
                        o_q += oun[hl, 0, :, sl] * np.exp(-s * ql)[None, :]
                    if qh < NQH - 1:
                        o_q += oun[hl, 2, :, sl] * \
                            np.exp(s * (ql - (QW - 1)))[None, :]
                else:
                    o_q = oun[hl, 1, :, sl]
                acc[:, sl] = o_q
            denom = acc[64, :]
            bvh = bv[Hg * 192 + 128:Hg * 192 + 192]  # V input-bias
            num = acc[:64, :] + bvh[:, None] * acc[65:66, :]
            o_h = (num / denom[None, :]) * maskf[None, :]
            out[b, :, Hg * HS:(Hg + 1) * HS] = o_h.T
    out += out_bias.reshape(1, 1, D)
    return out
